# revision 54
# baseline (speedup 1.0000x reference)
"""AudioCrossAttention on 8 Trainium2 NeuronCores.

Sharding: data-parallel over batch (B=2) x tensor-parallel over heads
(16 heads -> 4 heads / 256 dims per core).  Core c handles batch c//4 and
head-group c%4.  Each core computes its 4 heads' attention plus the partial
output projection over its 256-dim slice; partials are summed on the host
(the unshard step) and bo added there.

Everything on device flows in transposed layout ([d, s] / [skv, sq]) so no
transposes are ever needed:
  qT[d,sq]  = WqT.T @ xT          (lhsT=WqT [din,256], rhs=visual.T)
  kT[d,skv] = WkT.T @ xT (+bk +L-RoPE emb, fused into the PSUM eviction)
  v[skv,d]  = xT.T @ WvT  (natural layout, ones column appended per head)
  scoresT[skv,sq] = kT_h.T @ qT_h          per head, K=hd=64
  expT = exp(0.125 * scoresT)              (no max-subtract; scores are O(5))
  [outT; denom] = [v_h | 1].T @ expT       (ones column -> row 64 = denom)
  outT /= denom  (reciprocal -> gpsimd partition_broadcast -> DVE mult)
  finalT[e,sq] += WoT_c.T @ outT           (partial over this core's d-slice)

The K=64 scores matmuls only use half the PE contraction rows, so the two
heads of an mt-group (partitions 0-63 / 64-127) are emitted back-to-back:
their auto-derived tile_positions (0,0)/(64,0) put them on disjoint 64x128
row-tiles of the PE array and the hardware runs them concurrently (the
second LDWEIGHTS is pulled ahead by the PE reorder window).  Each pair
lands in one [128, 2*CH] PSUM tile -> a single [128, 1024] exp.

The v bias never appears on-device: sum(attn)=1 exactly, so it commutes to
a constant Wo@bv folded into bo on the host.

Matmul operands are fp16 (cast on host); accumulation stays fp32 in PSUM,
the projection partials leave the chip in fp16 and are summed fp32 on host.
"""

import sys

if '/opt/trn_rl_repo' not in sys.path:
    sys.path.insert(0, '/opt/trn_rl_repo')

import numpy as np

B = 2
SQ = 2048
SKV = 2048
DIM = 1024
NUM_HEADS = 16
HEAD_DIM = 64
N_CORES = 8
HPC = 4          # heads per core
DSL = 256        # d_out slice per core
CH = 512         # sq chunk width
NCH = SQ // CH   # 4
KT = DIM // 128  # 8  d_in k-tiles
ST = SKV // 128  # 16 skv tiles
SCALE = HEAD_DIM ** -0.5

_CACHE = {}


def _build():
    import concourse.bacc as bacc
    import concourse.mybir as mybir
    from concourse import tile

    F32 = mybir.dt.float32
    F16 = mybir.dt.float16
    AF = mybir.ActivationFunctionType
    ALU = mybir.AluOpType

    nc = bacc.Bacc("TRN2", target_bir_lowering=False, debug=False,
                   num_devices=N_CORES)

    # all tensors are pre-laid-out on the host so every DMA is contiguous
    # per partition (max descriptor size, min Sync descriptor-gen time)
    xq = nc.dram_tensor("xq", [128, NCH * 2 * 4 * CH], F16, kind="ExternalInput")
    xa = nc.dram_tensor("xa", [128, NCH * 2 * 4 * CH], F16, kind="ExternalInput")
    wk = nc.dram_tensor("wk", [128, KT * DSL], F16, kind="ExternalInput")
    wq = nc.dram_tensor("wq", [128, KT * DSL], F16, kind="ExternalInput")
    wv = nc.dram_tensor("wv", [128, KT * DSL], F16, kind="ExternalInput")
    wo = nc.dram_tensor("wo", [128, 2 * DIM], F16, kind="ExternalInput")
    emb2 = nc.dram_tensor("emb2", [128, SKV], F16, kind="ExternalInput")
    bqk2 = nc.dram_tensor("bqk2", [128, 4], F32, kind="ExternalInput")
    # output is partition-major [p, c, e, s] so the evict DMA is contiguous
    # per partition (the [e*128+p, s] layout scattered 1KB segments at 1MB
    # strides and ran at ~45GB/s, putting an 11us DMA on the kernel tail)
    out = nc.dram_tensor("out", [128, NCH * 8 * CH], F16, kind="ExternalOutput")

    with tile.TileContext(nc) as tc:
        with tc.tile_pool(name="consts", bufs=1) as consts, \
             tc.tile_pool(name="big", bufs=1) as big, \
             tc.tile_pool(name="xqp", bufs=2) as xqp, \
             tc.tile_pool(name="xap", bufs=4) as xap, \
             tc.tile_pool(name="expp", bufs=40) as expp, \
             tc.tile_pool(name="evp", bufs=1) as evp, \
             tc.tile_pool(name="smallp", bufs=2) as smallp, \
             tc.tile_pool(name="ps512", bufs=2, space="PSUM") as ps512, \
             tc.tile_pool(name="ps1024", bufs=2, space="PSUM") as ps1024, \
             tc.tile_pool(name="psav", bufs=2, space="PSUM") as psav:

            # ---- constants.  The k-side DMAs (wk halves interleaved with the
            # xa0 halves) go on the Sync DGE queue; the q-side (wq, xq0) and
            # small constants go on the Activation DGE queue, which is idle
            # until the first exp ~20us in.  Each DIRECT2D descriptor-gen
            # costs ~0.65us on its queue, so splitting halves the serial
            # issue chain in front of the first matmul.
            wk_sb = consts.tile([128, KT, DSL], F16, tag="wk")
            wk_r = wk.rearrange("p (kt m) -> p kt m", kt=KT)
            nc.sync.dma_start(out=wk_sb[:, 0:4, :], in_=wk_r[:, 0:4, :])
            wq_sb = consts.tile([128, KT, DSL], F16, tag="wq")
            nc.scalar.dma_start(out=wq_sb,
                                in_=wq.rearrange("p (kt m) -> p kt m", kt=KT))
            emb_sb = consts.tile([128, SKV], F16, tag="emb")
            bqk_sb = consts.tile([128, 4], F32, tag="bqk")
            bq_sb = bqk_sb[:, 0:2]
            bk_sb = bqk_sb[:, 2:4]
            wv_sb = consts.tile([128, KT, DSL], F16, tag="wv")
            wo_sb = consts.tile([128, 2, DIM], F16, tag="wo")

            onescol_f = consts.tile([128, ST * HPC], F32, tag="onescol")
            nc.vector.memset(onescol_f, 1.0)

            # ---- persistent activations ----
            qT = big.tile([128, 2, SQ], F16, tag="qT")
            kT = big.tile([128, 2, SKV], F16, tag="kT")
            oT0 = big.tile([128, SQ], F16, tag="oT0")
            oT1 = big.tile([128, SQ], F16, tag="oT1")
            oTs = [oT0, oT1]
            v4 = big.tile([128, ST, HPC, 68], F16, tag="v4")
            nc.vector.tensor_copy(
                v4[:, :, :, 64:65],
                onescol_f.rearrange("p (s g) -> p s g", s=ST).unsqueeze(3))

            # ---- software-pipelined emission ----
            # Everything below is ONE interleaved instruction stream: score
            # pairs are paced to the ACT exp rate, AV accumulation steps and
            # the out-projection ride in the gaps, so no engine FIFO ever
            # head-of-line blocks behind a PSUM tile that exp hasn't freed.
            st_ = {
                "si": 0,          # next score-order index
                "kT_chunks": [0, 0],   # per-mt: kT evicted through chunk-1
                "v_chunks": 0,
                "q_done": set(),       # (c, mt) pairs evicted
                "gi": 0,          # current AV group index
                "av_s2": 0,       # s2 progress within current AV group
                "pav": None,      # (tileA, tileB) for current AV group
                "out_c": 0,       # next chunk to out-project
                "out_e": 0,
                "oev": None,      # current out-evict SBUF tile
                "out_ready": set(),
            }
            et_store = {}   # (c, hp) -> {s2: et tile}
            scored = {}     # (c, hp) -> count of emitted s2 (in order)
            groups = [(c, hp) for c in range(NCH) for hp in range(2)]
            # score production order MUST equal AV consumption order
            # (group-major): the expp pool is a ring, so et slots free in
            # production order -- a mismatched order deadlocks across the
            # PE/ACT FIFOs.  The in-flight cap keeps the ring from wrapping
            # onto an et whose AV consumer hasn't been emitted yet.
            score_order = [(c, hp, s2) for c in range(NCH) for hp in range(2)
                           for s2 in range(ST)]
            ET_CAP = 38
            st_["av_steps"] = 0

            def can_score():
                if st_["si"] >= len(score_order):
                    return False
                c, hp, s2 = score_order[st_["si"]]
                # hp selects the mt half of kT/qT, so the gates are per-mt:
                # half a kproj/qproj unlocks the next batch of score pairs
                return ((c, hp) in st_["q_done"]
                        and s2 < 4 * st_["kT_chunks"][hp]
                        and st_["si"] - st_["av_steps"] < ET_CAP)

            def emit_score():
                c, hp, s2 = score_order[st_["si"]]
                st_["si"] += 1
                pss = ps1024.tile([128, 2 * CH], F32, tag="sc",
                                  name=f"pss{hp}_{c}_{s2}")
                for half in range(2):
                    pb = half * 64
                    nc.tensor.matmul(
                        pss[:, half * CH:(half + 1) * CH],
                        kT[pb:pb + 64, hp, s2 * 128:(s2 + 1) * 128],
                        qT[pb:pb + 64, hp, c * CH:(c + 1) * CH],
                        start=True, stop=True)
                et = expp.tile([128, 2 * CH], F16, tag="exp",
                               name=f"et{hp}_{c}_{s2}")
                nc.scalar.activation(et, pss, AF.Exp, scale=SCALE)
                et_store.setdefault((c, hp), {})[s2] = et
                scored[(c, hp)] = scored.get((c, hp), 0) + 1

            def _chain(h, pav, c):
                # exact DVE reciprocal is ~8 cycles/elem; the fast-approx
                # custom op (~18 bits, 5x faster) is plenty for softmax
                # denominators in [3e2, 3e5].  recip reads the PSUM denom row
                # directly (saves a 0.6us DVE copy on this latency chain).
                mt, pb = h // 2, (h % 2) * 64
                denrow = smallp.tile([1, CH], F32, tag="rec")
                nc.vector.tensor_copy(denrow, pav[64:65, :])
                drec = smallp.tile([1, CH], F32, tag="drec")
                nc.vector.reciprocal_approx_fast(drec, denrow)
                bc_sb = smallp.tile([64, CH], F32, tag="bcs")
                nc.gpsimd.partition_broadcast(bc_sb, drec)
                nc.vector.tensor_mul(oTs[mt][pb:pb + 64, c * CH:(c + 1) * CH],
                                     pav[0:64, :], bc_sb)

            def can_av(lag=0):
                if st_["gi"] >= len(groups):
                    return False
                c, hp = groups[st_["gi"]]
                s2 = st_["av_s2"]
                scores_left = st_["si"] < len(score_order)
                # A new group's first AV waits on the previous group's chain
                # (psav slot reuse, ~3-4us of recip/broadcast/mult).  Score
                # execution is locked to ACT pace by the 2-slot score-PSUM
                # ring, so the only way to keep ACT fed through that wait is
                # to emit ~4 score pairs BETWEEN the chain and this AV --
                # they execute at exp cadence while the chain drains.
                if (s2 == 0 and st_["gi"] > 0 and scores_left
                        and st_["si"] < st_.get("gate_si", 0)
                        and st_["si"] - st_["av_steps"] < ET_CAP):
                    return False
                # lag: stay several exps behind the score frontier so AV
                # matmuls never sit in the PE FIFO waiting on ACT semaphores
                return (s2 < scored.get((c, hp), 0)
                        and s2 < 4 * st_["v_chunks"]
                        and (lag == 0
                             or st_["si"] - st_["av_steps"] > lag
                             or not scores_left))

            def emit_av_step():
                # one s2 step = both heads of the current AV group
                c, hp = groups[st_["gi"]]
                s2 = st_["av_s2"]
                if st_["pav"] is None:
                    st_["pav"] = tuple(
                        psav.tile([128, CH], F32, tag="av",
                                  name=f"pav{c}_{hp}_{i}")
                        for i in range(2))
                et = et_store[(c, hp)][s2]
                for half in range(2):
                    h = 2 * hp + half
                    nc.tensor.matmul(
                        st_["pav"][half][0:65, :], v4[:, s2, h, 0:65],
                        et[:, half * CH:(half + 1) * CH],
                        start=(s2 == 0), stop=(s2 == ST - 1))
                st_["av_s2"] = s2 + 1
                st_["av_steps"] += 1
                if s2 == ST - 1:
                    for half in range(2):
                        _chain(2 * hp + half, st_["pav"][half], c)
                    st_["pav"] = None
                    st_["av_s2"] = 0
                    st_["gi"] += 1
                    st_["gate_si"] = st_["si"] + 4
                    if hp == 1:
                        st_["out_ready"].add(c)

            def can_out():
                return (st_["out_c"] < NCH
                        and st_["out_c"] in st_["out_ready"])

            def emit_out_e():
                c, e = st_["out_c"], st_["out_e"]
                if st_["oev"] is None:
                    st_["oev"] = evp.tile([128, 8, CH], F16, tag="ev",
                                          name=f"oev{c}")
                pso = ps512.tile([128, CH], F32, tag="mm", name=f"pso{c}_{e}")
                for kt in range(2):
                    nc.tensor.matmul(pso, wo_sb[:, kt, e * 128:(e + 1) * 128],
                                     oTs[kt][:, c * CH:(c + 1) * CH],
                                     start=(kt == 0), stop=(kt == 1))
                # evict in two half-casts: chain ops (recip/mult) slotting
                # into the DVE queue wait <=350ns instead of a full cast
                nc.vector.tensor_copy(st_["oev"][:, e, 0:CH // 2],
                                      pso[:, 0:CH // 2])
                nc.vector.tensor_copy(st_["oev"][:, e, CH // 2:CH],
                                      pso[:, CH // 2:CH])
                st_["out_e"] += 1
                if e % 2 == 1:
                    lo = e - 1
                    nc.sync.dma_start(
                        out=out.rearrange("p (c e s) -> p c e s", c=NCH, e=8)[
                            :, c, lo:e + 1, :],
                        in_=st_["oev"][:, lo:e + 1, :])
                if e == 7:
                    st_["oev"] = None
                    st_["out_e"] = 0
                    st_["out_c"] += 1

            def pump(ns=1, na=1, no=0):
                for _ in range(ns):
                    if can_score():
                        emit_score()
                # self-balancing AV: one step per score keeps ACT paced;
                # a second step only while the backlog exceeds ~14 so the
                # post-exp tail stays short without starving ACT
                if st_["si"] - st_["av_steps"] > 10:
                    na += 1
                for _ in range(na):
                    if can_av(lag=9):
                        emit_av_step()
                for _ in range(no):
                    if can_out():
                        emit_out_e()

            xa_ts, xq_ts = {}, {}
            xa_r = xa.rearrange("p (c h k s) -> p c h k s", c=NCH, h=2, k=4)
            xq_r = xq.rearrange("p (c h k s) -> p c h k s", c=NCH, h=2, k=4)

            def fetch_xa(c, eng=None):
                eng = eng or nc.sync
                xa_t = xap.tile([128, KT, CH], F16, tag="xa", name=f"xa{c}")
                xa_ts[c] = xa_t
                # two half-DMAs so the first matmul doesn't wait on 1MB
                for hlf in range(2):
                    eng.dma_start(out=xa_t[:, 4 * hlf:4 * hlf + 4, :],
                                  in_=xa_r[:, c, hlf])

            def fetch_xq(c, eng=None):
                eng = eng or nc.sync
                xq_t = xqp.tile([128, KT, CH], F16, tag="xq", name=f"xq{c}")
                xq_ts[c] = xq_t
                for hlf in range(2):
                    eng.dma_start(out=xq_t[:, 4 * hlf:4 * hlf + 4, :],
                                  in_=xq_r[:, c, hlf])

            def kproj(c, mts=(0, 1)):
                # mt-granular: each 8-matmul half evicts immediately and
                # opens its hp's score gate half a projection earlier
                xa_t = xa_ts[c]
                for mt in mts:
                    psk = ps512.tile([128, CH], F32, tag="mm", name=f"psk{c}_{mt}")
                    for kt in range(KT):
                        nc.tensor.matmul(psk, wk_sb[:, kt, mt * 128:(mt + 1) * 128],
                                         xa_t[:, kt, :], start=(kt == 0),
                                         stop=(kt == KT - 1))
                        pump(ns=1, na=1)
                    # kT = (psum + bk) + emb (emb rows duplicated across halves)
                    nc.vector.scalar_tensor_tensor(
                        kT[:, mt, c * CH:(c + 1) * CH], psk, bk_sb[:, mt:mt + 1],
                        emb_sb[:, c * CH:(c + 1) * CH], ALU.add, ALU.add)
                    st_["kT_chunks"][mt] = c + 1

            def qproj(c, mts=(0, 1)):
                xq_t = xq_ts[c]
                for mt in mts:
                    psq = ps512.tile([128, CH], F32, tag="mm", name=f"psq{c}_{mt}")
                    for kt in range(KT):
                        nc.tensor.matmul(psq, wq_sb[:, kt, mt * 128:(mt + 1) * 128],
                                         xq_t[:, kt, :], start=(kt == 0),
                                         stop=(kt == KT - 1))
                        pump(ns=1, na=1)
                    nc.vector.tensor_scalar_add(qT[:, mt, c * CH:(c + 1) * CH],
                                                psq, bq_sb[:, mt:mt + 1])
                    st_["q_done"].add((c, mt))

            def vproj(c):
                xa_t = xa_ts[c]
                for j in range(HPC):
                    stile = c * HPC + j
                    psv = ps512.tile([128, CH], F32, tag="mm", name=f"psv{c}_{j}")
                    for kt in range(KT):
                        nc.tensor.matmul(psv[:, 0:DSL], xa_t[:, kt, j * 128:(j + 1) * 128],
                                         wv_sb[:, kt, :], start=(kt == 0),
                                         stop=(kt == KT - 1))
                    nc.vector.tensor_copy(
                        v4[:, stile, :, 0:64],
                        psv[:, 0:DSL].rearrange("p (g m) -> p g m", g=HPC))
                    pump(ns=2, na=2, no=1 if j % 2 == 1 else 0)
                st_["v_chunks"] = c + 1

            # kproj is interleaved ahead of the q/v pipeline so the kT gates
            # open early and the group-major score stream never starves ACT.
            # Fetches are hoisted so the DMA queue stays ahead of compute;
            # deferred const DMAs slot in behind the x-chunks they'd delay.
            # The score stream is group-major: finishing group (c=0,hp=0)
            # needs ALL FOUR kT chunks, so every kproj is front-loaded --
            # otherwise ACT (the pacing engine: 135us of exp vs ~152us PE)
            # stalls ~10us per late kT chunk waiting for s2 gates to open.
            # sync queue: k-side (xa0h0, wk[4:], xa0h1 -- each wk half lands
            # just before the matmuls that need it); scalar queue: q-side
            # first, then the small eviction constants (needed ~2us later)
            # sync queue: k-side (wk/xa0 halves interleaved so each lands
            # just before the matmuls that need it); scalar queue: q-side
            # first, then the small eviction constants (needed ~2us later)
            xa_t0 = xap.tile([128, KT, CH], F16, tag="xa", name="xa0")
            xa_ts[0] = xa_t0
            nc.sync.dma_start(out=xa_t0[:, 0:4, :], in_=xa_r[:, 0, 0])
            fetch_xq(0, eng=nc.scalar)
            nc.sync.dma_start(out=wk_sb[:, 4:8, :], in_=wk_r[:, 4:8, :])
            nc.scalar.dma_start(out=emb_sb[:, 0:CH], in_=emb2[:, 0:CH])
            nc.sync.dma_start(out=xa_t0[:, 4:8, :], in_=xa_r[:, 0, 1])
            nc.scalar.dma_start(out=bqk_sb, in_=bqk2[:, :])
            # ACT exp-table prewarm: behind the critical scalar-queue DMAs
            # (a table load at the queue head would delay their issue), but
            # well before the first real exp
            warm_in = smallp.tile([1, 8], F32, tag="warm")
            nc.vector.memset(warm_in, 0.0)
            warm_out = smallp.tile([1, 8], F32, tag="warm2")
            nc.scalar.activation(warm_out, warm_in, AF.Exp, scale=1.0)
            fetch_xa(1)
            nc.scalar.dma_start(out=emb_sb[:, CH:SKV], in_=emb2[:, CH:SKV])
            # Deadline-driven ordering.  The exp stream (ACT, the pacing
            # engine) consumes score pairs group-major from ~t=27 at
            # 1.06us/pair; its gates are: kT chunks (all four, ASAP), the
            # v chunks (AV trails scores by ~6 pairs and a stalled AV fills
            # the et ring, which blocks scores at ET_CAP), and qT chunk c
            # just before pair 32c.  Hence k0 q0 k* v0 v1 q1 v2 v3 q2 q3.
            kproj(0, (0,))
            qproj(0, (0,))   # first score pair fires here, ~5us earlier
            kproj(0, (1,))
            kproj(1, (0,))
            qproj(0, (1,))
            fetch_xa(2)
            kproj(1, (1,))
            nc.sync.dma_start(out=wv_sb,
                              in_=wv.rearrange("p (kt m) -> p kt m", kt=KT))
            fetch_xa(3)
            fetch_xq(1)
            kproj(2)
            kproj(3)
            vproj(0)
            vproj(1)
            qproj(1)
            nc.sync.dma_start(out=wo_sb,
                              in_=wo.rearrange("p (kt m) -> p kt m", kt=2))
            fetch_xq(2)
            vproj(2)
            qproj(2)
            fetch_xq(3)
            vproj(3)
            qproj(3)

            # drain: remaining scores / AV / out-projection fully interleaved
            while (st_["si"] < len(score_order) or st_["gi"] < len(groups)
                   or st_["out_c"] < NCH):
                progressed = False
                scored_now = False
                if can_score():
                    emit_score()
                    progressed = scored_now = True
                scores_done = st_["si"] >= len(score_order)
                # AV normally tracks scores 1:1 (ACT is the pacer; >1 AV per
                # score starves it); run a second step while the backlog is
                # high, and catch up freely once scores are blocked or done
                if scores_done:
                    n_av = 4
                elif st_["si"] - st_["av_steps"] > 10 or not scored_now:
                    n_av = 2
                else:
                    n_av = 1
                for _ in range(n_av):
                    if can_av(lag=0 if scores_done else 9):
                        emit_av_step()
                        progressed = True
                for _ in range(3 if scores_done else 1):
                    if can_out():
                        emit_out_e()
                        progressed = True
                assert progressed, "emission pipeline stuck"

    nc.compile()
    return nc


def _make_runner(nc):
    """Build a reusable jitted SPMD executor (mirrors bass2jax.run_bass_via_pjrt)."""
    import jax
    import numpy as _np
    from jax.sharding import Mesh, PartitionSpec
    from jax.experimental.shard_map import shard_map
    import concourse.mybir as mybir
    from concourse.bass2jax import (_bass_exec_p, install_neuronx_cc_hook,
                                    partition_id_tensor)

    install_neuronx_cc_hook()
    partition_name = nc.partition_id_tensor.name if nc.partition_id_tensor else None

    in_names, out_names, out_avals, zero_outs = [], [], [], []
    for alloc in nc.m.functions[0].allocations:
        if not isinstance(alloc, mybir.MemoryLocationSet):
            continue
        name = alloc.memorylocations[0].name
        if alloc.kind == "ExternalInput":
            if name != partition_name:
                in_names.append(name)
        elif alloc.kind == "ExternalOutput":
            shape = tuple(alloc.tensor_shape)
            dtype = mybir.dt.np(alloc.dtype)
            out_names.append(name)
            out_avals.append(jax.core.ShapedArray(shape, dtype))
            zero_outs.append(_np.zeros(shape, dtype))
    n_params = len(in_names)
    n_outs = len(out_avals)
    all_in_names = list(in_names) + list(out_names)
    if partition_name is not None:
        all_in_names.append(partition_name)
    donate = tuple(range(n_params, n_params + n_outs))

    def _body(*args):
        operands = list(args)
        if partition_name is not None:
            operands.append(partition_id_tensor())
        outs = _bass_exec_p.bind(
            *operands,
            out_avals=tuple(out_avals),
            in_names=tuple(all_in_names),
            out_names=tuple(out_names),
            lowering_input_output_aliases=(),
            sim_require_finite=True,
            sim_require_nnan=True,
            nc=nc,
        )
        return tuple(outs)

    devices = jax.devices()[:N_CORES]
    mesh = Mesh(np.asarray(devices), ("core",))
    in_specs = (PartitionSpec("core"),) * (n_params + n_outs)
    out_specs = (PartitionSpec("core"),) * n_outs
    sharded = jax.jit(
        shard_map(_body, mesh=mesh, in_specs=in_specs, out_specs=out_specs,
                  check_rep=False),
        donate_argnums=donate, keep_unused=True)
    # non-donating variant for repeat-timing with device-resident operands
    sharded_nd = jax.jit(
        shard_map(_body, mesh=mesh, in_specs=in_specs, out_specs=out_specs,
                  check_rep=False),
        keep_unused=True)

    def _concat(in_maps):
        concat_in = [
            np.concatenate([np.asarray(in_maps[c][name]) for c in range(N_CORES)], axis=0)
            for name in in_names
        ]
        concat_zeros = [np.zeros((N_CORES * z.shape[0], *z.shape[1:]), z.dtype)
                        for z in zero_outs]
        return concat_in, concat_zeros

    def run(in_maps, unpack=True):
        concat_in, concat_zeros = _concat(in_maps)
        out_arrs = sharded(*concat_in, *concat_zeros)
        if not unpack:
            jax.block_until_ready(out_arrs)
            return None
        return [
            {name: np.asarray(out_arrs[i]).reshape(N_CORES, *out_avals[i].shape)[c]
             for i, name in enumerate(out_names)}
            for c in range(N_CORES)
        ]

    def stage(in_maps):
        """device_put all operands once; returns args for timed_call."""
        from jax.sharding import NamedSharding
        sh = NamedSharding(mesh, PartitionSpec("core"))
        concat_in, concat_zeros = _concat(in_maps)
        dev = [jax.device_put(x, sh) for x in concat_in + concat_zeros]
        jax.block_until_ready(dev)
        return dev

    def timed_call(dev_args):
        out_arrs = sharded_nd(*dev_args)
        jax.block_until_ready(out_arrs)
        return out_arrs

    run.stage = stage
    run.timed_call = timed_call
    return run


def _get_runner():
    if "runner" not in _CACHE:
        nc = _build()
        _CACHE["nc"] = nc
        _CACHE["runner"] = _make_runner(nc)
    return _CACHE["runner"]


def _lrope_embT(label_emb, labels):
    inv_freq = (1.0 / (10000.0 ** (np.arange(0, HEAD_DIM, 2, dtype=np.float32)
                                   / HEAD_DIM))).astype(np.float32)
    pos = np.arange(SKV, dtype=np.float32)
    freqs = np.outer(pos, inv_freq)
    emb = np.concatenate([np.sin(freqs), np.cos(freqs)], axis=-1).astype(np.float32)
    lab = np.asarray(label_emb, np.float32)[np.asarray(labels).astype(np.int64)]
    return emb, lab  # [SKV, HD], [B, HD]


def make_in_maps(visual_features, audio_features, audio_labels,
                 Wq, bq, Wk, bk, Wv, bv, Wo, bo, label_emb):
    vis = np.asarray(visual_features, np.float32)
    aud = np.asarray(audio_features, np.float32)
    Wq = np.asarray(Wq, np.float32)
    Wk = np.asarray(Wk, np.float32)
    Wv = np.asarray(Wv, np.float32)
    Wo = np.asarray(Wo, np.float32)
    bq = np.asarray(bq, np.float32)
    bk = np.asarray(bk, np.float32)
    emb, lab = _lrope_embT(label_emb, audio_labels)

    def xlay(x):
        # [S, DIM] -> [128, NCH, 2, 4, CH] chunk-major, contiguous per
        # partition for max-size DMA descriptors
        t = x.T.astype(np.float16).reshape(2, 4, 128, NCH, CH)  # [h,k,p,c,s]
        return np.ascontiguousarray(
            t.transpose(2, 3, 0, 1, 4).reshape(128, NCH * 2 * 4 * CH))

    def wlay(w):
        # [din=1024, m] -> [128, KT, m] p-major contiguous
        m = w.shape[1]
        t = w.astype(np.float16).reshape(KT, 128, m).transpose(1, 0, 2)
        return np.ascontiguousarray(t.reshape(128, KT * m))

    xqs = [xlay(vis[b]) for b in range(B)]
    xas = [xlay(aud[b]) for b in range(B)]
    embs = []
    for b in range(B):
        embT = np.ascontiguousarray((emb * lab[b][None, :]).T)  # [64, SKV]
        embs.append(np.concatenate([embT, embT], axis=0).astype(np.float16))

    in_maps = []
    for core in range(N_CORES):
        b, g = core // HPC, core % HPC
        sl = slice(g * DSL, (g + 1) * DSL)
        bqkc = np.concatenate(
            [bq[sl].reshape(2, 128).T, bk[sl].reshape(2, 128).T], axis=1)
        woc = Wo[:, sl].T.astype(np.float16).reshape(2, 128, DIM)
        in_maps.append({
            "xq": xqs[b],
            "xa": xas[b],
            "wk": wlay(Wk[sl, :].T),
            "wq": wlay(Wq[sl, :].T),
            "wv": wlay(Wv[sl, :].T),
            "wo": np.ascontiguousarray(
                woc.transpose(1, 0, 2).reshape(128, 2 * DIM)),
            "emb2": embs[b],
            "bqk2": np.ascontiguousarray(bqkc),
        })
    return in_maps


def kernel(**inputs):
    run = _get_runner()
    in_maps = make_in_maps(**inputs)
    results = run(in_maps)
    bo = np.asarray(inputs["bo"], np.float32)
    bv = np.asarray(inputs["bv"], np.float32)
    Wo = np.asarray(inputs["Wo"], np.float32)
    bo_eff = bo + Wo @ bv  # v-bias commutes through softmax (weights sum to 1)
    out = np.empty((B, SQ, DIM), np.float32)
    for b in range(B):
        s = results[4 * b]["out"].astype(np.float32)
        for g in range(1, HPC):
            s = s + results[4 * b + g]["out"].astype(np.float32)
        # device layout [p, c, e, s] -> [e*128+p, c*CH+s] = [DIM, SQ]
        s = s.reshape(128, NCH, 8, CH).transpose(2, 0, 1, 3).reshape(DIM, SQ)
        out[b] = s.T + bo_eff[None, :]
    return out



# revision 56
# speedup vs baseline: 1.0083x; 1.0083x over previous
"""AudioCrossAttention on 8 Trainium2 NeuronCores.

Sharding: data-parallel over batch (B=2) x tensor-parallel over heads
(16 heads -> 4 heads / 256 dims per core).  Core c handles batch c//4 and
head-group c%4.  Each core computes its 4 heads' attention plus the partial
output projection over its 256-dim slice; partials are summed on the host
(the unshard step) and bo added there.

Everything on device flows in transposed layout ([d, s] / [skv, sq]) so no
transposes are ever needed:
  qT[d,sq]  = WqT.T @ xT          (lhsT=WqT [din,256], rhs=visual.T)
  kT[d,skv] = WkT.T @ xT (+bk +L-RoPE emb, fused into the PSUM eviction)
  v[skv,d]  = xT.T @ WvT  (natural layout, ones column appended per head)
  scoresT[skv,sq] = kT_h.T @ qT_h          per head, K=hd=64
  expT = exp(0.125 * scoresT)              (no max-subtract; scores are O(5))
  [outT; denom] = [v_h | 1].T @ expT       (ones column -> row 64 = denom)
  outT /= denom  (reciprocal -> gpsimd partition_broadcast -> DVE mult)
  finalT[e,sq] += WoT_c.T @ outT           (partial over this core's d-slice)

The K=64 scores matmuls only use half the PE contraction rows, so the two
heads of an mt-group (partitions 0-63 / 64-127) are emitted back-to-back:
their auto-derived tile_positions (0,0)/(64,0) put them on disjoint 64x128
row-tiles of the PE array and the hardware runs them concurrently (the
second LDWEIGHTS is pulled ahead by the PE reorder window).  Each pair
lands in one [128, 2*CH] PSUM tile -> a single [128, 1024] exp.

The v bias never appears on-device: sum(attn)=1 exactly, so it commutes to
a constant Wo@bv folded into bo on the host.

Matmul operands are fp16 (cast on host); accumulation stays fp32 in PSUM,
the projection partials leave the chip in fp16 and are summed fp32 on host.
"""

import sys

if '/opt/trn_rl_repo' not in sys.path:
    sys.path.insert(0, '/opt/trn_rl_repo')

import numpy as np

B = 2
SQ = 2048
SKV = 2048
DIM = 1024
NUM_HEADS = 16
HEAD_DIM = 64
N_CORES = 8
HPC = 4          # heads per core
DSL = 256        # d_out slice per core
CH = 512         # sq chunk width
NCH = SQ // CH   # 4
KT = DIM // 128  # 8  d_in k-tiles
ST = SKV // 128  # 16 skv tiles
SCALE = HEAD_DIM ** -0.5

_CACHE = {}


def _build():
    import concourse.bacc as bacc
    import concourse.mybir as mybir
    from concourse import tile

    F32 = mybir.dt.float32
    F16 = mybir.dt.float16
    AF = mybir.ActivationFunctionType
    ALU = mybir.AluOpType

    nc = bacc.Bacc("TRN2", target_bir_lowering=False, debug=False,
                   num_devices=N_CORES)

    # all tensors are pre-laid-out on the host so every DMA is contiguous
    # per partition (max descriptor size, min Sync descriptor-gen time)
    xq = nc.dram_tensor("xq", [128, NCH * 2 * 4 * CH], F16, kind="ExternalInput")
    xa = nc.dram_tensor("xa", [128, NCH * 2 * 4 * CH], F16, kind="ExternalInput")
    wk = nc.dram_tensor("wk", [128, KT * DSL], F16, kind="ExternalInput")
    wq = nc.dram_tensor("wq", [128, KT * DSL], F16, kind="ExternalInput")
    wv = nc.dram_tensor("wv", [128, KT * DSL], F16, kind="ExternalInput")
    wo = nc.dram_tensor("wo", [128, 2 * DIM], F16, kind="ExternalInput")
    emb2 = nc.dram_tensor("emb2", [128, SKV], F16, kind="ExternalInput")
    bqk2 = nc.dram_tensor("bqk2", [128, 4], F32, kind="ExternalInput")
    # output is partition-major [p, c, e, s] so the evict DMA is contiguous
    # per partition (the [e*128+p, s] layout scattered 1KB segments at 1MB
    # strides and ran at ~45GB/s, putting an 11us DMA on the kernel tail)
    out = nc.dram_tensor("out", [128, NCH * 8 * CH], F16, kind="ExternalOutput")

    with tile.TileContext(nc) as tc:
        with tc.tile_pool(name="consts", bufs=1) as consts, \
             tc.tile_pool(name="big", bufs=1) as big, \
             tc.tile_pool(name="xqp", bufs=2) as xqp, \
             tc.tile_pool(name="xap", bufs=4) as xap, \
             tc.tile_pool(name="expp", bufs=40) as expp, \
             tc.tile_pool(name="evp", bufs=1) as evp, \
             tc.tile_pool(name="smallp", bufs=2) as smallp, \
             tc.tile_pool(name="ps512", bufs=2, space="PSUM") as ps512, \
             tc.tile_pool(name="ps1024", bufs=2, space="PSUM") as ps1024, \
             tc.tile_pool(name="psav", bufs=2, space="PSUM") as psav:

            # ---- constants.  The k-side DMAs (wk halves interleaved with the
            # xa0 halves) go on the Sync DGE queue; the q-side (wq, xq0) and
            # small constants go on the Activation DGE queue, which is idle
            # until the first exp ~20us in.  Each DIRECT2D descriptor-gen
            # costs ~0.65us on its queue, so splitting halves the serial
            # issue chain in front of the first matmul.
            wk_sb = consts.tile([128, KT, DSL], F16, tag="wk")
            wk_r = wk.rearrange("p (kt m) -> p kt m", kt=KT)
            nc.sync.dma_start(out=wk_sb[:, 0:4, :], in_=wk_r[:, 0:4, :])
            wq_sb = consts.tile([128, KT, DSL], F16, tag="wq")
            nc.scalar.dma_start(out=wq_sb,
                                in_=wq.rearrange("p (kt m) -> p kt m", kt=KT))
            emb_sb = consts.tile([128, SKV], F16, tag="emb")
            bqk_sb = consts.tile([128, 4], F32, tag="bqk")
            bq_sb = bqk_sb[:, 0:2]
            bk_sb = bqk_sb[:, 2:4]
            wv_sb = consts.tile([128, KT, DSL], F16, tag="wv")
            wo_sb = consts.tile([128, 2, DIM], F16, tag="wo")

            onescol_f = consts.tile([128, ST * HPC], F32, tag="onescol")
            nc.vector.memset(onescol_f, 1.0)

            # ---- PE p-state warmup: ~3us of dummy zero-matmuls while the
            # first DMAs land, so the real projections start at the ramped
            # clock (cold matmuls run 426-584ns vs 216ns steady) ----
            warm_a = consts.tile([128, CH], F16, tag="warma")
            nc.vector.memset(warm_a, 0.0)
            for wi in range(8):
                wps = ps512.tile([128, CH], F32, tag="mm", name=f"wps{wi}")
                nc.tensor.matmul(wps, warm_a[:, 0:128], warm_a,
                                 start=True, stop=True)

            # ---- persistent activations ----
            qT = big.tile([128, 2, SQ], F16, tag="qT")
            kT = big.tile([128, 2, SKV], F16, tag="kT")
            oT0 = big.tile([128, SQ], F16, tag="oT0")
            oT1 = big.tile([128, SQ], F16, tag="oT1")
            oTs = [oT0, oT1]
            v4 = big.tile([128, ST, HPC, 68], F16, tag="v4")
            nc.vector.tensor_copy(
                v4[:, :, :, 64:65],
                onescol_f.rearrange("p (s g) -> p s g", s=ST).unsqueeze(3))

            # ---- software-pipelined emission ----
            # Everything below is ONE interleaved instruction stream: score
            # pairs are paced to the ACT exp rate, AV accumulation steps and
            # the out-projection ride in the gaps, so no engine FIFO ever
            # head-of-line blocks behind a PSUM tile that exp hasn't freed.
            st_ = {
                "si": 0,          # next score-order index
                "kT_chunks": [0, 0],   # per-mt: kT evicted through chunk-1
                "v_chunks": 0,
                "q_done": set(),       # (c, mt) pairs evicted
                "gi": 0,          # current AV group index
                "av_s2": 0,       # s2 progress within current AV group
                "pav": None,      # (tileA, tileB) for current AV group
                "out_c": 0,       # next chunk to out-project
                "out_e": 0,
                "oev": None,      # current out-evict SBUF tile
                "out_ready": set(),
            }
            et_store = {}   # (c, hp) -> {s2: et tile}
            scored = {}     # (c, hp) -> count of emitted s2 (in order)
            groups = [(c, hp) for c in range(NCH) for hp in range(2)]
            # score production order MUST equal AV consumption order
            # (group-major): the expp pool is a ring, so et slots free in
            # production order -- a mismatched order deadlocks across the
            # PE/ACT FIFOs.  The in-flight cap keeps the ring from wrapping
            # onto an et whose AV consumer hasn't been emitted yet.
            score_order = [(c, hp, s2) for c in range(NCH) for hp in range(2)
                           for s2 in range(ST)]
            ET_CAP = 38
            st_["av_steps"] = 0

            def can_score():
                if st_["si"] >= len(score_order):
                    return False
                c, hp, s2 = score_order[st_["si"]]
                # hp selects the mt half of kT/qT, so the gates are per-mt:
                # half a kproj/qproj unlocks the next batch of score pairs
                return ((c, hp) in st_["q_done"]
                        and s2 < 4 * st_["kT_chunks"][hp]
                        and st_["si"] - st_["av_steps"] < ET_CAP)

            def emit_score():
                c, hp, s2 = score_order[st_["si"]]
                st_["si"] += 1
                pss = ps1024.tile([128, 2 * CH], F32, tag="sc",
                                  name=f"pss{hp}_{c}_{s2}")
                for half in range(2):
                    pb = half * 64
                    nc.tensor.matmul(
                        pss[:, half * CH:(half + 1) * CH],
                        kT[pb:pb + 64, hp, s2 * 128:(s2 + 1) * 128],
                        qT[pb:pb + 64, hp, c * CH:(c + 1) * CH],
                        start=True, stop=True)
                et = expp.tile([128, 2 * CH], F16, tag="exp",
                               name=f"et{hp}_{c}_{s2}")
                nc.scalar.activation(et, pss, AF.Exp, scale=SCALE)
                et_store.setdefault((c, hp), {})[s2] = et
                scored[(c, hp)] = scored.get((c, hp), 0) + 1

            def _chain(h, pav, c):
                # exact DVE reciprocal is ~8 cycles/elem; the fast-approx
                # custom op (~18 bits, 5x faster) is plenty for softmax
                # denominators in [3e2, 3e5].  recip reads the PSUM denom row
                # directly (saves a 0.6us DVE copy on this latency chain).
                mt, pb = h // 2, (h % 2) * 64
                denrow = smallp.tile([1, CH], F32, tag="rec")
                nc.vector.tensor_copy(denrow, pav[64:65, :])
                drec = smallp.tile([1, CH], F32, tag="drec")
                nc.vector.reciprocal_approx_fast(drec, denrow)
                bc_sb = smallp.tile([64, CH], F32, tag="bcs")
                nc.gpsimd.partition_broadcast(bc_sb, drec)
                nc.vector.tensor_mul(oTs[mt][pb:pb + 64, c * CH:(c + 1) * CH],
                                     pav[0:64, :], bc_sb)

            def can_av(lag=0):
                if st_["gi"] >= len(groups):
                    return False
                c, hp = groups[st_["gi"]]
                s2 = st_["av_s2"]
                scores_left = st_["si"] < len(score_order)
                # A new group's first AV waits on the previous group's chain
                # (psav slot reuse, ~3-4us of recip/broadcast/mult).  Score
                # execution is locked to ACT pace by the 2-slot score-PSUM
                # ring, so the only way to keep ACT fed through that wait is
                # to emit ~6 score pairs BETWEEN the chain and this AV --
                # they execute at exp cadence while the chain drains.
                if (s2 == 0 and st_["gi"] > 0 and scores_left
                        and st_["si"] < st_.get("gate_si", 0)
                        and st_["si"] - st_["av_steps"] < ET_CAP):
                    return False
                # lag: stay several exps behind the score frontier so AV
                # matmuls never sit in the PE FIFO waiting on ACT semaphores
                return (s2 < scored.get((c, hp), 0)
                        and s2 < 4 * st_["v_chunks"]
                        and (lag == 0
                             or st_["si"] - st_["av_steps"] > lag
                             or not scores_left))

            def emit_av_step():
                # one s2 step = both heads of the current AV group
                c, hp = groups[st_["gi"]]
                s2 = st_["av_s2"]
                if st_["pav"] is None:
                    st_["pav"] = tuple(
                        psav.tile([128, CH], F32, tag="av",
                                  name=f"pav{c}_{hp}_{i}")
                        for i in range(2))
                et = et_store[(c, hp)][s2]
                for half in range(2):
                    h = 2 * hp + half
                    nc.tensor.matmul(
                        st_["pav"][half][0:65, :], v4[:, s2, h, 0:65],
                        et[:, half * CH:(half + 1) * CH],
                        start=(s2 == 0), stop=(s2 == ST - 1))
                st_["av_s2"] = s2 + 1
                st_["av_steps"] += 1
                if s2 == ST - 1:
                    for half in range(2):
                        _chain(2 * hp + half, st_["pav"][half], c)
                    st_["pav"] = None
                    st_["av_s2"] = 0
                    st_["gi"] += 1
                    st_["gate_si"] = st_["si"] + 6
                    if hp == 1:
                        st_["out_ready"].add(c)

            def can_out():
                return (st_["out_c"] < NCH
                        and st_["out_c"] in st_["out_ready"])

            def emit_out_e():
                c, e = st_["out_c"], st_["out_e"]
                if st_["oev"] is None:
                    st_["oev"] = evp.tile([128, 8, CH], F16, tag="ev",
                                          name=f"oev{c}")
                pso = ps512.tile([128, CH], F32, tag="mm", name=f"pso{c}_{e}")
                for kt in range(2):
                    nc.tensor.matmul(pso, wo_sb[:, kt, e * 128:(e + 1) * 128],
                                     oTs[kt][:, c * CH:(c + 1) * CH],
                                     start=(kt == 0), stop=(kt == 1))
                # evict in two half-casts: chain ops (recip/mult) slotting
                # into the DVE queue wait <=350ns instead of a full cast
                nc.vector.tensor_copy(st_["oev"][:, e, 0:CH // 2],
                                      pso[:, 0:CH // 2])
                nc.vector.tensor_copy(st_["oev"][:, e, CH // 2:CH],
                                      pso[:, CH // 2:CH])
                st_["out_e"] += 1
                if e % 2 == 1:
                    lo = e - 1
                    nc.sync.dma_start(
                        out=out.rearrange("p (c e s) -> p c e s", c=NCH, e=8)[
                            :, c, lo:e + 1, :],
                        in_=st_["oev"][:, lo:e + 1, :])
                if e == 7:
                    st_["oev"] = None
                    st_["out_e"] = 0
                    st_["out_c"] += 1

            def pump(ns=1, na=1, no=0):
                for _ in range(ns):
                    if can_score():
                        emit_score()
                # self-balancing AV: one step per score keeps ACT paced;
                # a second step only while the backlog exceeds ~14 so the
                # post-exp tail stays short without starving ACT
                if st_["si"] - st_["av_steps"] > 10:
                    na += 1
                for _ in range(na):
                    if can_av(lag=9):
                        emit_av_step()
                for _ in range(no):
                    if can_out():
                        emit_out_e()

            xa_ts, xq_ts = {}, {}
            xa_r = xa.rearrange("p (c h k s) -> p c h k s", c=NCH, h=2, k=4)
            xq_r = xq.rearrange("p (c h k s) -> p c h k s", c=NCH, h=2, k=4)

            def fetch_xa(c, eng=None):
                eng = eng or nc.sync
                xa_t = xap.tile([128, KT, CH], F16, tag="xa", name=f"xa{c}")
                xa_ts[c] = xa_t
                # two half-DMAs so the first matmul doesn't wait on 1MB
                for hlf in range(2):
                    eng.dma_start(out=xa_t[:, 4 * hlf:4 * hlf + 4, :],
                                  in_=xa_r[:, c, hlf])

            def fetch_xq(c, eng=None):
                eng = eng or nc.sync
                xq_t = xqp.tile([128, KT, CH], F16, tag="xq", name=f"xq{c}")
                xq_ts[c] = xq_t
                for hlf in range(2):
                    eng.dma_start(out=xq_t[:, 4 * hlf:4 * hlf + 4, :],
                                  in_=xq_r[:, c, hlf])

            def kproj(c, mts=(0, 1)):
                # mt-granular: each 8-matmul half evicts immediately and
                # opens its hp's score gate half a projection earlier
                xa_t = xa_ts[c]
                for mt in mts:
                    psk = ps512.tile([128, CH], F32, tag="mm", name=f"psk{c}_{mt}")
                    for kt in range(KT):
                        nc.tensor.matmul(psk, wk_sb[:, kt, mt * 128:(mt + 1) * 128],
                                         xa_t[:, kt, :], start=(kt == 0),
                                         stop=(kt == KT - 1))
                        pump(ns=1, na=1)
                    # kT = (psum + bk) + emb (emb rows duplicated across halves)
                    nc.vector.scalar_tensor_tensor(
                        kT[:, mt, c * CH:(c + 1) * CH], psk, bk_sb[:, mt:mt + 1],
                        emb_sb[:, c * CH:(c + 1) * CH], ALU.add, ALU.add)
                    st_["kT_chunks"][mt] = c + 1

            def qproj(c, mts=(0, 1)):
                xq_t = xq_ts[c]
                for mt in mts:
                    psq = ps512.tile([128, CH], F32, tag="mm", name=f"psq{c}_{mt}")
                    for kt in range(KT):
                        nc.tensor.matmul(psq, wq_sb[:, kt, mt * 128:(mt + 1) * 128],
                                         xq_t[:, kt, :], start=(kt == 0),
                                         stop=(kt == KT - 1))
                        pump(ns=1, na=1)
                    nc.vector.tensor_scalar_add(qT[:, mt, c * CH:(c + 1) * CH],
                                                psq, bq_sb[:, mt:mt + 1])
                    st_["q_done"].add((c, mt))

            def vproj(c):
                xa_t = xa_ts[c]
                for j in range(HPC):
                    stile = c * HPC + j
                    psv = ps512.tile([128, CH], F32, tag="mm", name=f"psv{c}_{j}")
                    for kt in range(KT):
                        nc.tensor.matmul(psv[:, 0:DSL], xa_t[:, kt, j * 128:(j + 1) * 128],
                                         wv_sb[:, kt, :], start=(kt == 0),
                                         stop=(kt == KT - 1))
                    nc.vector.tensor_copy(
                        v4[:, stile, :, 0:64],
                        psv[:, 0:DSL].rearrange("p (g m) -> p g m", g=HPC))
                    pump(ns=2, na=2, no=1 if j % 2 == 1 else 0)
                st_["v_chunks"] = c + 1

            # kproj is interleaved ahead of the q/v pipeline so the kT gates
            # open early and the group-major score stream never starves ACT.
            # Fetches are hoisted so the DMA queue stays ahead of compute;
            # deferred const DMAs slot in behind the x-chunks they'd delay.
            # The score stream is group-major: finishing group (c=0,hp=0)
            # needs ALL FOUR kT chunks, so every kproj is front-loaded --
            # otherwise ACT (the pacing engine: 135us of exp vs ~152us PE)
            # stalls ~10us per late kT chunk waiting for s2 gates to open.
            # sync queue: k-side (xa0h0, wk[4:], xa0h1 -- each wk half lands
            # just before the matmuls that need it); scalar queue: q-side
            # first, then the small eviction constants (needed ~2us later)
            # sync queue: k-side (wk/xa0 halves interleaved so each lands
            # just before the matmuls that need it); scalar queue: q-side
            # first, then the small eviction constants (needed ~2us later)
            xa_t0 = xap.tile([128, KT, CH], F16, tag="xa", name="xa0")
            xa_ts[0] = xa_t0
            nc.sync.dma_start(out=xa_t0[:, 0:4, :], in_=xa_r[:, 0, 0])
            fetch_xq(0, eng=nc.scalar)
            nc.sync.dma_start(out=wk_sb[:, 4:8, :], in_=wk_r[:, 4:8, :])
            nc.scalar.dma_start(out=emb_sb[:, 0:CH], in_=emb2[:, 0:CH])
            nc.sync.dma_start(out=xa_t0[:, 4:8, :], in_=xa_r[:, 0, 1])
            nc.scalar.dma_start(out=bqk_sb, in_=bqk2[:, :])
            # ACT exp-table prewarm: behind the critical scalar-queue DMAs
            # (a table load at the queue head would delay their issue), but
            # well before the first real exp
            warm_in = smallp.tile([1, 8], F32, tag="warm")
            nc.vector.memset(warm_in, 0.0)
            warm_out = smallp.tile([1, 8], F32, tag="warm2")
            nc.scalar.activation(warm_out, warm_in, AF.Exp, scale=1.0)
            fetch_xa(1)
            nc.scalar.dma_start(out=emb_sb[:, CH:SKV], in_=emb2[:, CH:SKV])
            # Deadline-driven ordering.  The exp stream (ACT, the pacing
            # engine) consumes score pairs group-major from ~t=27 at
            # 1.06us/pair; its gates are: kT chunks (all four, ASAP), the
            # v chunks (AV trails scores by ~6 pairs and a stalled AV fills
            # the et ring, which blocks scores at ET_CAP), and qT chunk c
            # just before pair 32c.  Hence k0 q0 k* v0 v1 q1 v2 v3 q2 q3.
            kproj(0, (0,))
            qproj(0, (0,))   # first score pair fires here, ~5us earlier
            kproj(0, (1,))
            kproj(1, (0,))
            qproj(0, (1,))
            fetch_xa(2)
            kproj(1, (1,))
            nc.sync.dma_start(out=wv_sb,
                              in_=wv.rearrange("p (kt m) -> p kt m", kt=KT))
            fetch_xa(3)
            fetch_xq(1)
            kproj(2)
            kproj(3)
            vproj(0)
            vproj(1)
            qproj(1)
            nc.sync.dma_start(out=wo_sb,
                              in_=wo.rearrange("p (kt m) -> p kt m", kt=2))
            fetch_xq(2)
            vproj(2)
            qproj(2)
            fetch_xq(3)
            vproj(3)
            qproj(3)

            # drain: remaining scores / AV / out-projection fully interleaved
            while (st_["si"] < len(score_order) or st_["gi"] < len(groups)
                   or st_["out_c"] < NCH):
                progressed = False
                scored_now = False
                if can_score():
                    emit_score()
                    progressed = scored_now = True
                scores_done = st_["si"] >= len(score_order)
                # AV normally tracks scores 1:1 (ACT is the pacer; >1 AV per
                # score starves it); run a second step while the backlog is
                # high, and catch up freely once scores are blocked or done
                if scores_done:
                    n_av = 4
                elif st_["si"] - st_["av_steps"] > 10 or not scored_now:
                    n_av = 2
                else:
                    n_av = 1
                for _ in range(n_av):
                    if can_av(lag=0 if scores_done else 9):
                        emit_av_step()
                        progressed = True
                for _ in range(3 if scores_done else 1):
                    if can_out():
                        emit_out_e()
                        progressed = True
                assert progressed, "emission pipeline stuck"

    nc.compile()
    return nc


def _make_runner(nc):
    """Build a reusable jitted SPMD executor (mirrors bass2jax.run_bass_via_pjrt)."""
    import jax
    import numpy as _np
    from jax.sharding import Mesh, PartitionSpec
    from jax.experimental.shard_map import shard_map
    import concourse.mybir as mybir
    from concourse.bass2jax import (_bass_exec_p, install_neuronx_cc_hook,
                                    partition_id_tensor)

    install_neuronx_cc_hook()
    partition_name = nc.partition_id_tensor.name if nc.partition_id_tensor else None

    in_names, out_names, out_avals, zero_outs = [], [], [], []
    for alloc in nc.m.functions[0].allocations:
        if not isinstance(alloc, mybir.MemoryLocationSet):
            continue
        name = alloc.memorylocations[0].name
        if alloc.kind == "ExternalInput":
            if name != partition_name:
                in_names.append(name)
        elif alloc.kind == "ExternalOutput":
            shape = tuple(alloc.tensor_shape)
            dtype = mybir.dt.np(alloc.dtype)
            out_names.append(name)
            out_avals.append(jax.core.ShapedArray(shape, dtype))
            zero_outs.append(_np.zeros(shape, dtype))
    n_params = len(in_names)
    n_outs = len(out_avals)
    all_in_names = list(in_names) + list(out_names)
    if partition_name is not None:
        all_in_names.append(partition_name)
    donate = tuple(range(n_params, n_params + n_outs))

    def _body(*args):
        operands = list(args)
        if partition_name is not None:
            operands.append(partition_id_tensor())
        outs = _bass_exec_p.bind(
            *operands,
            out_avals=tuple(out_avals),
            in_names=tuple(all_in_names),
            out_names=tuple(out_names),
            lowering_input_output_aliases=(),
            sim_require_finite=True,
            sim_require_nnan=True,
            nc=nc,
        )
        return tuple(outs)

    devices = jax.devices()[:N_CORES]
    mesh = Mesh(np.asarray(devices), ("core",))
    in_specs = (PartitionSpec("core"),) * (n_params + n_outs)
    out_specs = (PartitionSpec("core"),) * n_outs
    sharded = jax.jit(
        shard_map(_body, mesh=mesh, in_specs=in_specs, out_specs=out_specs,
                  check_rep=False),
        donate_argnums=donate, keep_unused=True)
    # non-donating variant for repeat-timing with device-resident operands
    sharded_nd = jax.jit(
        shard_map(_body, mesh=mesh, in_specs=in_specs, out_specs=out_specs,
                  check_rep=False),
        keep_unused=True)

    def _concat(in_maps):
        concat_in = [
            np.concatenate([np.asarray(in_maps[c][name]) for c in range(N_CORES)], axis=0)
            for name in in_names
        ]
        concat_zeros = [np.zeros((N_CORES * z.shape[0], *z.shape[1:]), z.dtype)
                        for z in zero_outs]
        return concat_in, concat_zeros

    def run(in_maps, unpack=True):
        concat_in, concat_zeros = _concat(in_maps)
        out_arrs = sharded(*concat_in, *concat_zeros)
        if not unpack:
            jax.block_until_ready(out_arrs)
            return None
        return [
            {name: np.asarray(out_arrs[i]).reshape(N_CORES, *out_avals[i].shape)[c]
             for i, name in enumerate(out_names)}
            for c in range(N_CORES)
        ]

    def stage(in_maps):
        """device_put all operands once; returns args for timed_call."""
        from jax.sharding import NamedSharding
        sh = NamedSharding(mesh, PartitionSpec("core"))
        concat_in, concat_zeros = _concat(in_maps)
        dev = [jax.device_put(x, sh) for x in concat_in + concat_zeros]
        jax.block_until_ready(dev)
        return dev

    def timed_call(dev_args):
        out_arrs = sharded_nd(*dev_args)
        jax.block_until_ready(out_arrs)
        return out_arrs

    run.stage = stage
    run.timed_call = timed_call
    return run


def _get_runner():
    if "runner" not in _CACHE:
        nc = _build()
        _CACHE["nc"] = nc
        _CACHE["runner"] = _make_runner(nc)
    return _CACHE["runner"]


def _lrope_embT(label_emb, labels):
    inv_freq = (1.0 / (10000.0 ** (np.arange(0, HEAD_DIM, 2, dtype=np.float32)
                                   / HEAD_DIM))).astype(np.float32)
    pos = np.arange(SKV, dtype=np.float32)
    freqs = np.outer(pos, inv_freq)
    emb = np.concatenate([np.sin(freqs), np.cos(freqs)], axis=-1).astype(np.float32)
    lab = np.asarray(label_emb, np.float32)[np.asarray(labels).astype(np.int64)]
    return emb, lab  # [SKV, HD], [B, HD]


def make_in_maps(visual_features, audio_features, audio_labels,
                 Wq, bq, Wk, bk, Wv, bv, Wo, bo, label_emb):
    vis = np.asarray(visual_features, np.float32)
    aud = np.asarray(audio_features, np.float32)
    Wq = np.asarray(Wq, np.float32)
    Wk = np.asarray(Wk, np.float32)
    Wv = np.asarray(Wv, np.float32)
    Wo = np.asarray(Wo, np.float32)
    bq = np.asarray(bq, np.float32)
    bk = np.asarray(bk, np.float32)
    emb, lab = _lrope_embT(label_emb, audio_labels)

    def xlay(x):
        # [S, DIM] -> [128, NCH, 2, 4, CH] chunk-major, contiguous per
        # partition for max-size DMA descriptors
        t = x.T.astype(np.float16).reshape(2, 4, 128, NCH, CH)  # [h,k,p,c,s]
        return np.ascontiguousarray(
            t.transpose(2, 3, 0, 1, 4).reshape(128, NCH * 2 * 4 * CH))

    def wlay(w):
        # [din=1024, m] -> [128, KT, m] p-major contiguous
        m = w.shape[1]
        t = w.astype(np.float16).reshape(KT, 128, m).transpose(1, 0, 2)
        return np.ascontiguousarray(t.reshape(128, KT * m))

    xqs = [xlay(vis[b]) for b in range(B)]
    xas = [xlay(aud[b]) for b in range(B)]
    embs = []
    for b in range(B):
        embT = np.ascontiguousarray((emb * lab[b][None, :]).T)  # [64, SKV]
        embs.append(np.concatenate([embT, embT], axis=0).astype(np.float16))

    in_maps = []
    for core in range(N_CORES):
        b, g = core // HPC, core % HPC
        sl = slice(g * DSL, (g + 1) * DSL)
        bqkc = np.concatenate(
            [bq[sl].reshape(2, 128).T, bk[sl].reshape(2, 128).T], axis=1)
        woc = Wo[:, sl].T.astype(np.float16).reshape(2, 128, DIM)
        in_maps.append({
            "xq": xqs[b],
            "xa": xas[b],
            "wk": wlay(Wk[sl, :].T),
            "wq": wlay(Wq[sl, :].T),
            "wv": wlay(Wv[sl, :].T),
            "wo": np.ascontiguousarray(
                woc.transpose(1, 0, 2).reshape(128, 2 * DIM)),
            "emb2": embs[b],
            "bqk2": np.ascontiguousarray(bqkc),
        })
    return in_maps


def kernel(**inputs):
    run = _get_runner()
    in_maps = make_in_maps(**inputs)
    results = run(in_maps)
    bo = np.asarray(inputs["bo"], np.float32)
    bv = np.asarray(inputs["bv"], np.float32)
    Wo = np.asarray(inputs["Wo"], np.float32)
    bo_eff = bo + Wo @ bv  # v-bias commutes through softmax (weights sum to 1)
    out = np.empty((B, SQ, DIM), np.float32)
    for b in range(B):
        s = results[4 * b]["out"].astype(np.float32)
        for g in range(1, HPC):
            s = s + results[4 * b + g]["out"].astype(np.float32)
        # device layout [p, c, e, s] -> [e*128+p, c*CH+s] = [DIM, SQ]
        s = s.reshape(128, NCH, 8, CH).transpose(2, 0, 1, 3).reshape(DIM, SQ)
        out[b] = s.T + bo_eff[None, :]
    return out

